# revision 16
# baseline (speedup 1.0000x reference)
"""Trainium2 Bass kernel for the quirky-reshape MultiHeadSelfAttention layer.

Reference math (B=1, S=2048, H=768):
    qkv = x @ W_qkv                  # (S, 2304)
    col c' = h*36 + t ; h in [0,64) "heads", t in [0,36): q=t<12, k=12<=t<24, v=t>=24
    per head h (d_k=12):  A_h = softmax(Q_h K_h^T / sqrt(12));  O_h = A_h V_h
    values[:, h*12+d] = O_h[:, d];   out = values @ W_o

Sharding: 8 heads per core (tensor-parallel over the 64-head axis).
Each core computes a rank-96 partial of the output projection; the host
sums the 8 partials (the "all-reduce on output" done at unshard time).

Per-core device pipeline:
  * QKV projection (fp16 operands, fp32 PSUM) with host-prepacked weights
    so Q^T/K^T land at the row-tile base partitions (32*s) needed for
    small-K(=12) matmul packing, and V lands as [j, 13] blocks with a
    built-in ones column (attention@V then also produces the softmax
    denominator D).
  * Attention per (i-chunk 512, j-block 128, head-group {3,3,2}):
    transposed logits A^T[j,i] via row-tiled fp16 matmuls into a 3-bank
    PSUM span, one Exp ACTIVATE over the span (1/sqrt(12) folded into the
    ACT pre-scale), fp16 exp weights, then attention@V col-tiled
    4-heads-per-PSUM-bank with per-element PSUM accumulation over j.
  * Everything QKV is emitted just-in-time INSIDE chunk 0's attention
    stream (single shared PSUM pool) so ACT starts ~immediately and the
    PE stays dense (HAM warm). Each chunk's softmax normalization +
    output projection rides in the next chunk's attention shadow.
  * Normalization: selector-matmul broadcasts each head's denominator to
    its whole 32-row group, fast reciprocal, one multiply; output
    projection uses host-prepacked zero-padded W_o rows (float32r).
"""

import numpy as np

import concourse.bass as bass
import concourse.mybir as mybir
import concourse.tile as tile
from concourse import bacc
from concourse.bass_utils import run_bass_kernel_spmd

F32 = mybir.dt.float32
F32R = mybir.dt.float32r
FP16 = mybir.dt.float16

S = 2048
H = 768
DK = 12            # per-head dim (reference N_HEADS)
HEADS_PER_CORE = 8
N_CORES = 8
SCALE = 1.0 / float(np.sqrt(DK))
# logits head groups: (group, n_slots); slots at base partitions 0/32/64
GROUPS = ((0, 3), (1, 3), (2, 2))
NGROUPS_PER_JB = len(GROUPS)
GROUPS_PER_IC = 16 * NGROUPS_PER_JB  # 48


def _head_of(g, s):
    return 3 * g + s if g < 2 else 6 + s


def build_program():
    nc = bacc.Bacc("TRN2", target_bir_lowering=False, debug=False)

    xt_d = nc.dram_tensor("xt", [H, S], FP16, kind="ExternalInput")
    wqk_d = nc.dram_tensor("wqk", [H, 2, 3, 128], FP16, kind="ExternalInput")
    wv_d = nc.dram_tensor("wv", [H, 96], FP16, kind="ExternalInput")
    wo_d = nc.dram_tensor("wo", [2, 128, H], F32R, kind="ExternalInput")
    sel_d = nc.dram_tensor("sel", [128, 128], F32R, kind="ExternalInput")
    out_d = nc.dram_tensor("out", [S, H], F32, kind="ExternalOutput")

    with tile.TileContext(nc) as tc:
        with tc.tile_pool(name="const", bufs=1) as cpool, \
             tc.tile_pool(name="ps_l", bufs=2, space="PSUM") as ps_l, \
             tc.tile_pool(name="ps_av", bufs=1, space="PSUM") as ps_av, \
             tc.tile_pool(name="esb", bufs=6) as esb, \
             tc.tile_pool(name="osb", bufs=2) as opool:
            xt = cpool.tile([128, 6, S], FP16, tag="xt")
            wqk = cpool.tile([128, 6, 2, 3, 128], FP16, tag="wqk")
            wv = cpool.tile([128, 6, 96], FP16, tag="wv")
            wo = cpool.tile([128, 2, H], F32R, tag="wo")
            sel = cpool.tile([128, 128], F32R, tag="sel")
            qkt = cpool.tile([128, 2, 3, S], FP16, tag="qkt")
            vsb = cpool.tile([128, 16, 8, 13], FP16, tag="vsb")
            vhat = cpool.tile([128, 2, S], F32R, tag="vhat")

            # interleave weight/activation loads; first QKV group needs all
            # six h-blocks, so just split for DMA-queue parallelism
            xt_r = xt_d.rearrange("(hb p) s -> p hb s", p=128)
            wqk_r = wqk_d.rearrange("(hb p) t g m -> p hb t g m", p=128)
            for hb in range(6):
                nc.sync.dma_start(wqk[:, hb], wqk_r[:, hb])
                nc.sync.dma_start(xt[:, hb, :], xt_r[:, hb, :])
            nc.sync.dma_start(wv[:], wv_d.rearrange("(hb p) n -> p hb n", p=128))
            nc.sync.dma_start(wo[:], wo_d.rearrange("b p o -> p b o"))
            nc.sync.dma_start(sel[:], sel_d[:])
            zscratch = cpool.tile([128, S], F32, tag="zscratch")
            nc.vector.memset(zscratch[:], 0.0)
            for b in range(2):
                nc.vector.tensor_copy(vhat[:, b, :], zscratch[:])
            # ones column (index 12) for the denominator trick; V columns
            # 0..11 get overwritten by emit_v below.
            nc.vector.memset(vsb[:], 1.0)

            # ---- QKV producers (PSUM via shared L-slots) ----
            def emit_qk(t, g, ch):
                p = ps_l.tile([128, 3, 512], F32, tag="L", name=f"pqk{t}{g}{ch}")
                for hb in range(6):
                    nc.tensor.matmul(
                        p[:, 0, :],
                        lhsT=wqk[:, hb, t, g, :],
                        rhs=xt[:, hb, ch * 512:(ch + 1) * 512],
                        start=(hb == 0),
                        stop=(hb == 5),
                    )
                nc.vector.tensor_copy(qkt[:, t, g, ch * 512:(ch + 1) * 512],
                                      p[:, 0, :])

            def emit_v(sb):
                p = ps_l.tile([128, 3, 512], F32, tag="L", name=f"pv{sb}")
                for hb in range(6):
                    nc.tensor.matmul(
                        p[:, 0, :96],
                        lhsT=xt[:, hb, sb * 128:(sb + 1) * 128],
                        rhs=wv[:, hb, :],
                        start=(hb == 0),
                        stop=(hb == 5),
                    )
                nc.vector.tensor_copy(
                    vsb[:, sb, :, 0:12],
                    p[:, 0, :96].rearrange("p (h d) -> p h d", d=12),
                )

            # ---- per-chunk epilogue steps (normalize + project + store) ----
            def emit_bc(ic, b):
                lo, hi = ic * 512, (ic + 1) * 512
                t_ = ps_l.tile([128, 3, 512], F32, tag="L", name=f"bc{b}_{ic}")
                bc = t_[:, 0, :]
                nc.tensor.matmul(bc, lhsT=sel[:], rhs=vhat[:, b, lo:hi],
                                 start=True, stop=True)
                nc.vector.reciprocal_approx_fast(bc, bc)
                nc.vector.tensor_tensor(vhat[:, b, lo:hi], vhat[:, b, lo:hi],
                                        bc, mybir.AluOpType.mult)

            def emit_po(ib):
                t_ = ps_l.tile([128, 3, 512], F32, tag="L", name=f"po_{ib}")
                for b in range(2):
                    nc.tensor.matmul(
                        t_[:, 0, :],
                        lhsT=vhat[:, b, ib * 128:(ib + 1) * 128],
                        rhs=wo[:, b, 0:512],
                        start=(b == 0), stop=(b == 1),
                    )
                for b in range(2):
                    nc.tensor.matmul(
                        t_[:, 1, :256],
                        lhsT=vhat[:, b, ib * 128:(ib + 1) * 128],
                        rhs=wo[:, b, 512:768],
                        start=(b == 0), stop=(b == 1),
                    )
                osb = opool.tile([128, 768], F32, tag="osb", name=f"osb_{ib}")
                nc.vector.tensor_copy(osb[:, 0:512], t_[:, 0, :])
                nc.vector.tensor_copy(osb[:, 512:768], t_[:, 1, :256])
                nc.sync.dma_start(out_d[ib * 128:(ib + 1) * 128, :], osb[:])

            def emit_av(av, E, g, nslots, jb):
                for s in range(nslots):
                    h = _head_of(g, s)
                    b, c = divmod(h, 4)
                    # has_written tracking is per-partition, so the four
                    # col-slots of one bank are independent accumulation
                    # groups (disjoint partitions).
                    nc.tensor.matmul(
                        av[b][32 * c:32 * c + 13, :],
                        lhsT=vsb[:, jb, h, :],
                        rhs=E[:, s, :],
                        start=(jb == 0),
                        stop=(jb == 15),
                        tile_position=(0, 32 * c),
                        # CoreSim's group checker is not partition-aware;
                        # the pending-zero numerics are.
                        skip_group_check=True,
                    )

            # ---- injection schedules ----
            # chunk 0 carries the remaining QKV work just-in-time:
            #   logits(jb,*) need K(*, jb//4); av(jb,*) need V(sb=jb);
            #   Q(*, ch>0) are only needed from chunk 1 on.
            sched = {ic: {} for ic in range(4)}

            def add(ic, gidx, thunk):
                sched[ic].setdefault(max(0, gidx), []).append(thunk)

            for ch in (1, 2, 3):
                for gi, g in enumerate(range(3)):
                    add(0, 12 * ch - 7 + gi, (lambda t=1, g=g, ch=ch:
                                              emit_qk(t, g, ch)))
            for sb in range(2, 16):
                add(0, 3 * sb - 4, (lambda sb=sb: emit_v(sb)))
            for ci, ch in enumerate((1, 2, 3)):
                for gi, g in enumerate(range(3)):
                    add(0, 33 + 5 * ci + gi, (lambda t=0, g=g, ch=ch:
                                              emit_qk(t, g, ch)))
            # chunks 1..3 carry the previous chunk's epilogue
            for ic in (1, 2, 3):
                add(ic, 0, (lambda ic=ic: emit_bc(ic - 1, 0)))
                add(ic, 1, (lambda ic=ic: emit_bc(ic - 1, 1)))
                for k in range(4):
                    add(ic, 2 + k, (lambda ic=ic, k=k: emit_po((ic - 1) * 4 + k)))

            # ---- prologue: the minimum QKV needed to start chunk 0 ----
            for g in range(3):
                emit_qk(1, g, 0)      # K^T group g, first j-span
            for g in range(3):
                emit_qk(0, g, 0)      # Q^T group g, chunk 0
            emit_v(0)
            emit_v(1)

            # ---- attention ----
            for ic in range(4):
                av = None
                # one-group software-pipeline skew: emit each group's
                # attention@V after the NEXT group's logits, so the PE
                # never stalls on the Exp it just fed.
                pending = None
                gidx = 0
                for jb in range(16):
                    for g, nslots in GROUPS:
                        L = ps_l.tile([128, 3, 512], F32, tag="L",
                                      name=f"L_{ic}_{jb}_{g}")
                        for s in range(nslots):
                            nc.tensor.matmul(
                                L[:, s, :],
                                lhsT=qkt[32 * s:32 * s + 12, 1, g,
                                         jb * 128:(jb + 1) * 128],
                                rhs=qkt[32 * s:32 * s + 12, 0, g,
                                        ic * 512:(ic + 1) * 512],
                                start=True,
                                stop=True,
                                tile_position=(32 * s, 0),
                            )
                        E = esb.tile([128, 3, 512], FP16, tag="E")
                        nc.scalar.activation(
                            E[:, :nslots, :],
                            L[:, :nslots, :],
                            mybir.ActivationFunctionType.Exp,
                            scale=SCALE,
                        )
                        for thunk in sched[ic].get(gidx, ()):
                            thunk()
                        if pending is not None:
                            if av is None:
                                av = [ps_av.tile([128, 512], F32, tag=f"av{b}",
                                                 name=f"av{b}_{ic}")
                                      for b in range(2)]
                            emit_av(av, *pending)
                        pending = (E, g, nslots, jb)
                        gidx += 1
                emit_av(av, *pending)
                for b in range(2):
                    for c in range(4):
                        nc.vector.tensor_copy(
                            vhat[32 * c:32 * c + 13, b, ic * 512:(ic + 1) * 512],
                            av[b][32 * c:32 * c + 13, :],
                        )
            # last chunk's epilogue
            emit_bc(3, 0)
            emit_bc(3, 1)
            for k in range(4):
                emit_po(12 + k)

    nc.compile()
    return nc


def make_core_inputs(x, W_qkv, W_o):
    """Host-side shard/prepack. Returns list of per-core input dicts."""
    x = np.asarray(x, np.float32)
    W_qkv = np.asarray(W_qkv, np.float32)
    W_o = np.asarray(W_o, np.float32)
    xt = np.ascontiguousarray(x.reshape(S, H).T).astype(np.float16)  # [H, S]

    sel = np.zeros((128, 128), np.float32)
    for s4 in range(4):
        sel[32 * s4 + 12, 32 * s4:32 * (s4 + 1)] = 1.0

    in_maps = []
    for core in range(N_CORES):
        wqk = np.zeros((H, 2, 3, 128), np.float16)
        wv = np.zeros((H, 96), np.float16)
        wo = np.zeros((2, 128, H), np.float32)
        for g, nslots in GROUPS:
            for s in range(nslots):
                h = _head_of(g, s)
                Hg = HEADS_PER_CORE * core + h
                for t in range(2):
                    wqk[:, t, g, 32 * s:32 * s + 12] = \
                        W_qkv[:, Hg * 36 + t * 12:Hg * 36 + (t + 1) * 12]
        for h in range(HEADS_PER_CORE):
            Hg = HEADS_PER_CORE * core + h
            wv[:, 12 * h:12 * (h + 1)] = W_qkv[:, Hg * 36 + 24:Hg * 36 + 36]
            b, c = divmod(h, 4)
            wo[b, 32 * c:32 * c + 12, :] = W_o[Hg * DK:(Hg + 1) * DK, :]
        in_maps.append({"xt": xt, "wqk": wqk, "wv": wv, "wo": wo, "sel": sel})
    return in_maps


_NC_CACHE = None


def kernel(x, W_qkv, W_o):
    global _NC_CACHE
    if _NC_CACHE is None:
        _NC_CACHE = build_program()
    nc = _NC_CACHE
    in_maps = make_core_inputs(x, W_qkv, W_o)
    res = run_bass_kernel_spmd(nc, in_maps, core_ids=list(range(N_CORES)))
    out = np.zeros((S, H), np.float64)
    for r in res.results:
        out += r["out"].astype(np.float64)
    return out.astype(np.float32).reshape(1, S, H)


# revision 18
# speedup vs baseline: 1.2948x; 1.2948x over previous
"""Trainium2 Bass kernel for the quirky-reshape MultiHeadSelfAttention layer.

Reference math (B=1, S=2048, H=768):
    qkv = x @ W_qkv                  # (S, 2304)
    col c' = h*36 + t ; h in [0,64) "heads", t in [0,36): q=t<12, k=12<=t<24, v=t>=24
    per head h (d_k=12):  A_h = softmax(Q_h K_h^T / sqrt(12));  O_h = A_h V_h
    values[:, h*12+d] = O_h[:, d];   out = values @ W_o

Sharding: 8 heads per core (tensor-parallel over the 64-head axis).
Each core computes a rank-96 partial of the output projection; the host
sums the 8 partials (the "all-reduce on output" done at unshard time).

Per-core device pipeline:
  * QKV projection (fp16 operands, fp32 PSUM) with host-prepacked weights
    so Q^T/K^T land at the row-tile base partitions (32*s) needed for
    small-K(=12) matmul packing, and V lands as [j, 13] blocks with a
    built-in ones column (attention@V then also produces the softmax
    denominator D).
  * Attention per (i-chunk 512, j-block 128, head-group {3,3,2}):
    transposed logits A^T[j,i] via row-tiled fp16 matmuls into a 3-bank
    PSUM span, one Exp ACTIVATE over the span (1/sqrt(12) folded into the
    ACT pre-scale), fp16 exp weights, then attention@V col-tiled
    4-heads-per-PSUM-bank with per-element PSUM accumulation over j.
  * Everything QKV is emitted just-in-time INSIDE chunk 0's attention
    stream (single shared PSUM pool) so ACT starts ~immediately and the
    PE stays dense (HAM warm). Each chunk's softmax normalization +
    output projection rides in the next chunk's attention shadow.
  * Normalization: selector-matmul broadcasts each head's denominator to
    its whole 32-row group, fast reciprocal, one multiply; output
    projection uses host-prepacked zero-padded W_o rows (float32r).
"""

import numpy as np

import concourse.bass as bass
import concourse.mybir as mybir
import concourse.tile as tile
from concourse import bacc
from concourse.bass_utils import run_bass_kernel_spmd

F32 = mybir.dt.float32
F32R = mybir.dt.float32r
FP16 = mybir.dt.float16

S = 2048
H = 768
DK = 12            # per-head dim (reference N_HEADS)
HEADS_PER_CORE = 8
N_CORES = 8
SCALE = 1.0 / float(np.sqrt(DK))
# logits head groups: (group, n_slots); slots at base partitions 0/32/64
GROUPS = ((0, 3), (1, 3), (2, 2))
NGROUPS_PER_JB = len(GROUPS)
GROUPS_PER_IC = 16 * NGROUPS_PER_JB  # 48


def _head_of(g, s):
    return 3 * g + s if g < 2 else 6 + s


def build_program():
    nc = bacc.Bacc("TRN2", target_bir_lowering=False, debug=False)

    xt_d = nc.dram_tensor("xt", [H, S], FP16, kind="ExternalInput")
    wqk_d = nc.dram_tensor("wqk", [H, 2, 3, 128], FP16, kind="ExternalInput")
    wv_d = nc.dram_tensor("wv", [H, 96], FP16, kind="ExternalInput")
    wo_d = nc.dram_tensor("wo", [2, 128, H], F32R, kind="ExternalInput")
    sel_d = nc.dram_tensor("sel", [128, 128], F32R, kind="ExternalInput")
    out_d = nc.dram_tensor("out", [S, H], F32, kind="ExternalOutput")

    with tile.TileContext(nc) as tc:
        with tc.tile_pool(name="const", bufs=1) as cpool, \
             tc.tile_pool(name="ps_l", bufs=2, space="PSUM") as ps_l, \
             tc.tile_pool(name="ps_av", bufs=1, space="PSUM") as ps_av, \
             tc.tile_pool(name="esb", bufs=9) as esb, \
             tc.tile_pool(name="osb", bufs=2) as opool:
            xt = cpool.tile([128, 6, S], FP16, tag="xt")
            wqk = cpool.tile([128, 6, 2, 3, 128], FP16, tag="wqk")
            wv = cpool.tile([128, 6, 96], FP16, tag="wv")
            wo = cpool.tile([128, 2, H], F32R, tag="wo")
            sel = cpool.tile([128, 128], F32R, tag="sel")
            qkt = cpool.tile([128, 2, 3, S], FP16, tag="qkt")
            vsb = cpool.tile([128, 16, 8, 13], FP16, tag="vsb")
            vhat = cpool.tile([128, 2, S], F32R, tag="vhat")

            # interleave weight/activation loads; first QKV group needs all
            # six h-blocks, so just split for DMA-queue parallelism
            xt_r = xt_d.rearrange("(hb p) s -> p hb s", p=128)
            wqk_r = wqk_d.rearrange("(hb p) t g m -> p hb t g m", p=128)
            for hb in range(6):
                nc.sync.dma_start(wqk[:, hb], wqk_r[:, hb])
                nc.sync.dma_start(xt[:, hb, :], xt_r[:, hb, :])
            nc.sync.dma_start(wv[:], wv_d.rearrange("(hb p) n -> p hb n", p=128))
            nc.sync.dma_start(wo[:], wo_d.rearrange("b p o -> p b o"))
            nc.sync.dma_start(sel[:], sel_d[:])
            zscratch = cpool.tile([128, S], F32, tag="zscratch")
            nc.vector.memset(zscratch[:], 0.0)
            for b in range(2):
                nc.vector.tensor_copy(vhat[:, b, :], zscratch[:])
            # ones column (index 12) for the denominator trick; V columns
            # 0..11 get overwritten by emit_v below.
            nc.vector.memset(vsb[:], 1.0)

            # ---- QKV producers (PSUM via shared L-slots) ----
            def emit_qk(t, g, ch):
                p = ps_l.tile([128, 3, 512], F32, tag="L", name=f"pqk{t}{g}{ch}")
                for hb in range(6):
                    nc.tensor.matmul(
                        p[:, 0, :],
                        lhsT=wqk[:, hb, t, g, :],
                        rhs=xt[:, hb, ch * 512:(ch + 1) * 512],
                        start=(hb == 0),
                        stop=(hb == 5),
                    )
                nc.vector.tensor_copy(qkt[:, t, g, ch * 512:(ch + 1) * 512],
                                      p[:, 0, :])

            def emit_v(sb):
                p = ps_l.tile([128, 3, 512], F32, tag="L", name=f"pv{sb}")
                for hb in range(6):
                    nc.tensor.matmul(
                        p[:, 0, :96],
                        lhsT=xt[:, hb, sb * 128:(sb + 1) * 128],
                        rhs=wv[:, hb, :],
                        start=(hb == 0),
                        stop=(hb == 5),
                    )
                nc.vector.tensor_copy(
                    vsb[:, sb, :, 0:12],
                    p[:, 0, :96].rearrange("p (h d) -> p h d", d=12),
                )

            # ---- per-chunk epilogue steps (normalize + project + store) ----
            def emit_bc(ic, b):
                # runs in the av slot of its quad, before the next chunk's
                # av accumulators are allocated
                lo, hi = ic * 512, (ic + 1) * 512
                bc = ps_av.tile([128, 512], F32, tag=f"av{b}", name=f"bc{b}_{ic}")
                nc.tensor.matmul(bc[:], lhsT=sel[:], rhs=vhat[:, b, lo:hi],
                                 start=True, stop=True)
                nc.vector.reciprocal_approx_fast(bc[:], bc[:])
                nc.vector.tensor_tensor(vhat[:, b, lo:hi], vhat[:, b, lo:hi],
                                        bc[:], mybir.AluOpType.mult)

            def emit_po(ib):
                poa = ps_av.tile([128, 512], F32, tag="av0", name=f"poa_{ib}")
                pob = ps_av.tile([128, 512], F32, tag="av1", name=f"pob_{ib}")
                for b in range(2):
                    nc.tensor.matmul(
                        poa[:],
                        lhsT=vhat[:, b, ib * 128:(ib + 1) * 128],
                        rhs=wo[:, b, 0:512],
                        start=(b == 0), stop=(b == 1),
                    )
                for b in range(2):
                    nc.tensor.matmul(
                        pob[:, :256],
                        lhsT=vhat[:, b, ib * 128:(ib + 1) * 128],
                        rhs=wo[:, b, 512:768],
                        start=(b == 0), stop=(b == 1),
                    )
                osb = opool.tile([128, 768], F32, tag="osb", name=f"osb_{ib}")
                nc.vector.tensor_copy(osb[:, 0:512], poa[:])
                nc.vector.tensor_copy(osb[:, 512:768], pob[:, :256])
                nc.sync.dma_start(out_d[ib * 128:(ib + 1) * 128, :], osb[:])

            def emit_av(av, E, g, nslots, jb):
                for s in range(nslots):
                    h = _head_of(g, s)
                    b, c = divmod(h, 4)
                    # has_written tracking is per-partition, so the four
                    # col-slots of one bank are independent accumulation
                    # groups (disjoint partitions).
                    nc.tensor.matmul(
                        av[b][32 * c:32 * c + 13, :],
                        lhsT=vsb[:, jb, h, :],
                        rhs=E[:, s, :],
                        start=(jb == 0),
                        stop=(jb == 15),
                        tile_position=(0, 32 * c),
                        # CoreSim's group checker is not partition-aware;
                        # the pending-zero numerics are.
                        skip_group_check=True,
                    )

            # ---- injection schedules ----
            # chunk 0 carries the remaining QKV work just-in-time:
            #   logits(jb,*) need K(*, jb//4); av(jb,*) need V(sb=jb);
            #   Q(*, ch>0) are only needed from chunk 1 on.
            sched = {ic: {} for ic in range(4)}

            def add(ic, gidx, thunk):
                sched[ic].setdefault(max(0, gidx), []).append(thunk)

            # Q chunks 1..3 are only needed from the matching chunk on;
            # spread their (L-slot) injections thinly across earlier chunks.
            for ci, ch in enumerate((1, 2, 3)):
                for gi, g in enumerate(range(3)):
                    add(ci, 20 + 3 * gi, (lambda t=0, g=g, ch=ch:
                                          emit_qk(t, g, ch)))
            # chunks 1..3 carry the previous chunk's epilogue in the av
            # slots (idle between accumulation rounds; the deferred av
            # allocation below keeps the slot order correct).
            for ic in (1, 2, 3):
                add(ic, 0, (lambda ic=ic: emit_bc(ic - 1, 0)))
                add(ic, 1, (lambda ic=ic: emit_bc(ic - 1, 1)))
                for k in range(4):
                    add(ic, 2 + k, (lambda ic=ic, k=k: emit_po((ic - 1) * 4 + k)))

            # ---- prologue: K^T fully, Q^T chunk 0, V fully ----
            for ch in range(4):
                for g in range(3):
                    emit_qk(1, g, ch)
            for g in range(3):
                emit_qk(0, g, 0)
            for sb in range(16):
                emit_v(sb)

            # ---- attention ----
            from collections import deque
            LAG = 6
            for ic in range(4):
                av = None
                # deep software-pipeline skew: emit each group's
                # attention@V several groups after its Exp, so the PE never
                # stalls on ACT and the epilogue thunks (which reuse the av
                # slots) land before the chunk's own av allocation.
                pending = deque()
                gidx = 0
                for jb in range(16):
                    for g, nslots in GROUPS:
                        L = ps_l.tile([128, 3, 512], F32, tag="L",
                                      name=f"L_{ic}_{jb}_{g}")
                        for s in range(nslots):
                            nc.tensor.matmul(
                                L[:, s, :],
                                lhsT=qkt[32 * s:32 * s + 12, 1, g,
                                         jb * 128:(jb + 1) * 128],
                                rhs=qkt[32 * s:32 * s + 12, 0, g,
                                        ic * 512:(ic + 1) * 512],
                                start=True,
                                stop=True,
                                tile_position=(32 * s, 0),
                            )
                        E = esb.tile([128, 3, 512], FP16, tag="E")
                        nc.scalar.activation(
                            E[:, :nslots, :],
                            L[:, :nslots, :],
                            mybir.ActivationFunctionType.Exp,
                            scale=SCALE,
                        )
                        for thunk in sched[ic].get(gidx, ()):
                            thunk()
                        pending.append((E, g, nslots, jb))
                        if len(pending) > LAG:
                            if av is None:
                                av = [ps_av.tile([128, 512], F32, tag=f"av{b}",
                                                 name=f"av{b}_{ic}")
                                      for b in range(2)]
                            emit_av(av, *pending.popleft())
                        gidx += 1
                while pending:
                    emit_av(av, *pending.popleft())
                for b in range(2):
                    for c in range(4):
                        nc.vector.tensor_copy(
                            vhat[32 * c:32 * c + 13, b, ic * 512:(ic + 1) * 512],
                            av[b][32 * c:32 * c + 13, :],
                        )
            # last chunk's epilogue
            emit_bc(3, 0)
            emit_bc(3, 1)
            for k in range(4):
                emit_po(12 + k)

    nc.compile()
    return nc


def make_core_inputs(x, W_qkv, W_o):
    """Host-side shard/prepack. Returns list of per-core input dicts."""
    x = np.asarray(x, np.float32)
    W_qkv = np.asarray(W_qkv, np.float32)
    W_o = np.asarray(W_o, np.float32)
    xt = np.ascontiguousarray(x.reshape(S, H).T).astype(np.float16)  # [H, S]

    sel = np.zeros((128, 128), np.float32)
    for s4 in range(4):
        sel[32 * s4 + 12, 32 * s4:32 * (s4 + 1)] = 1.0

    in_maps = []
    for core in range(N_CORES):
        wqk = np.zeros((H, 2, 3, 128), np.float16)
        wv = np.zeros((H, 96), np.float16)
        wo = np.zeros((2, 128, H), np.float32)
        for g, nslots in GROUPS:
            for s in range(nslots):
                h = _head_of(g, s)
                Hg = HEADS_PER_CORE * core + h
                for t in range(2):
                    wqk[:, t, g, 32 * s:32 * s + 12] = \
                        W_qkv[:, Hg * 36 + t * 12:Hg * 36 + (t + 1) * 12]
        for h in range(HEADS_PER_CORE):
            Hg = HEADS_PER_CORE * core + h
            wv[:, 12 * h:12 * (h + 1)] = W_qkv[:, Hg * 36 + 24:Hg * 36 + 36]
            b, c = divmod(h, 4)
            wo[b, 32 * c:32 * c + 12, :] = W_o[Hg * DK:(Hg + 1) * DK, :]
        in_maps.append({"xt": xt, "wqk": wqk, "wv": wv, "wo": wo, "sel": sel})
    return in_maps


_NC_CACHE = None


def kernel(x, W_qkv, W_o):
    global _NC_CACHE
    if _NC_CACHE is None:
        _NC_CACHE = build_program()
    nc = _NC_CACHE
    in_maps = make_core_inputs(x, W_qkv, W_o)
    res = run_bass_kernel_spmd(nc, in_maps, core_ids=list(range(N_CORES)))
    out = np.zeros((S, H), np.float64)
    for r in res.results:
        out += r["out"].astype(np.float64)
    return out.astype(np.float32).reshape(1, S, H)
